# revision 1
# baseline (speedup 1.0000x reference)
"""CondMobileNetV2 forward kernel for nn_CondMobileNetV2_83476984365405.

Self-contained: takes FULL unsharded inputs (x [32,3,224,224] + params pytree),
returns FULL logits [32, 1000] float32.

Strategy: data-parallel over the batch axis across the 8 NeuronCores when the
accelerator path is available (jax shard_map, BN batch-stat all-reduce via
psum); falls back to a single-device / CPU jax execution of the identical
math if the device path fails for any reason. The math below mirrors the
torch CondMobileNetV2 semantics exactly (BatchNorm in training mode with
biased batch variance, CondConv per-sample expert mixing, relu6).
"""

import os
import numpy as np

import jax
import jax.numpy as jnp

# ---------------------------------------------------------------- arch config
CFGS = [[1, 16, 1, 1], [6, 24, 2, 2], [6, 32, 3, 2], [6, 64, 4, 2],
        [6, 96, 3, 1], [6, 160, 3, 2], [6, 320, 1, 1]]
NUM_EXPERTS = 8
NUM_CLASSES = 1000
BATCH = 32
N_CORES = 8


def _make_divisible(v, divisor=8):
    new_v = max(divisor, int(v + divisor / 2) // divisor * divisor)
    if new_v < 0.9 * v:
        new_v += divisor
    return new_v


def block_specs(width_mult=1.0):
    specs = []
    inp = _make_divisible(32 * width_mult)
    ne = None
    for j, (t, c, n, s) in enumerate(CFGS):
        out = _make_divisible(c * width_mult)
        for i in range(n):
            specs.append((inp, out, s if i == 0 else 1, t, ne))
            inp = out
            if j == 4 and i == 0:
                ne = NUM_EXPERTS
    return specs, inp


# ---------------------------------------------------------------- model math
def conv(x, w, stride, pad, groups=1):
    return jax.lax.conv_general_dilated(
        x, w, (stride, stride), [(pad, pad), (pad, pad)],
        feature_group_count=groups, dimension_numbers=("NCHW", "OIHW", "NCHW"))


def cond_conv(x, w, rw, stride, pad, groups=1):
    # w: [K, Cout, Cin//groups, kh, kw], rw: [B, K]  -> per-sample mixed weights
    cw = jnp.einsum("bk,koihw->boihw", rw, w)

    def per_sample(xi, wi):
        return conv(xi[None], wi, stride, pad, groups)[0]

    return jax.vmap(per_sample)(x, cw)


def bn_from_stats(x, mean, var, g, b, eps=1e-5):
    return ((x - mean[None, :, None, None])
            * jax.lax.rsqrt(var[None, :, None, None] + eps)
            * g[None, :, None, None] + b[None, :, None, None])


def make_forward(axis_name=None):
    """Builds the forward fn. If axis_name is set, BN batch statistics are
    all-reduced across the data-parallel axis (shard_map over batch)."""

    def batch_stats(x):
        # torch BatchNorm2d training-mode forward: biased batch stats over
        # (N, H, W). Cross-core: average of per-core (mean, mean-of-sq).
        m = x.mean((0, 2, 3))
        msq = (x * x).mean((0, 2, 3))
        if axis_name is not None:
            m = jax.lax.pmean(m, axis_name)
            msq = jax.lax.pmean(msq, axis_name)
        return m, msq - m * m

    def bn(x, g, b):
        m, v = batch_stats(x)
        return bn_from_stats(x, m, v, g, b)

    def relu6(x):
        return jnp.clip(x, 0.0, 6.0)

    def block_fwd(x, d, spec):
        inp, oup, s, t, ne = spec
        h = x
        if ne:
            rw = jax.nn.sigmoid(x.mean((2, 3)) @ d["route_w"].T + d["route_b"])
            if t != 1:
                h = relu6(bn(cond_conv(h, d["pw_w"], rw, 1, 0), d["pw_g"], d["pw_b"]))
            hid = d["dw_w"].shape[1]
            h = relu6(bn(cond_conv(h, d["dw_w"], rw, s, 1, groups=hid),
                         d["dw_g"], d["dw_b"]))
            h = bn(cond_conv(h, d["pl_w"], rw, 1, 0), d["pl_g"], d["pl_b"])
        else:
            if t != 1:
                h = relu6(bn(conv(h, d["pw_w"], 1, 0), d["pw_g"], d["pw_b"]))
            hid = d["dw_w"].shape[0]
            h = relu6(bn(conv(h, d["dw_w"], s, 1, groups=hid), d["dw_g"], d["dw_b"]))
            h = bn(conv(h, d["pl_w"], 1, 0), d["pl_g"], d["pl_b"])
        return h + x if (s == 1 and inp == oup) else h

    def forward(x, params):
        specs, _ = block_specs()
        h = relu6(bn(conv(x, params["stem_w"], 2, 1), params["stem_g"], params["stem_b"]))
        for d, spec in zip(params["blocks"], specs):
            h = block_fwd(h, d, spec)
        h = relu6(bn(conv(h, params["head_w"], 1, 0), params["head_g"], params["head_b"]))
        pooled = h.mean((2, 3))  # [B_local, 1280]
        rw = jax.nn.sigmoid(pooled @ params["cls_route_w"].T + params["cls_route_b"])
        cw = jnp.einsum("bk,koc->boc", rw, params["cls_w"][..., 0, 0])
        logits = jnp.einsum("bc,boc->bo", pooled, cw)
        return logits

    return forward


# ---------------------------------------------------------------- entrypoints
def _to_np_tree(t):
    if isinstance(t, dict):
        return {k: _to_np_tree(v) for k, v in t.items()}
    if isinstance(t, (list, tuple)):
        return type(t)(_to_np_tree(v) for v in t)
    return np.asarray(t, dtype=np.float32)


def _run_sharded_device(x, params):
    """Data-parallel across 8 NeuronCores: batch split 4/core, weights
    replicated, BN stats pmean-all-reduced (the only cross-core comms)."""
    from jax.sharding import Mesh, PartitionSpec as P
    from jax.experimental.shard_map import shard_map

    devs = jax.devices()
    if len(devs) < N_CORES:
        raise RuntimeError(f"need {N_CORES} devices, have {len(devs)}")
    mesh = Mesh(np.asarray(devs[:N_CORES]), ("b",))
    fwd = make_forward(axis_name="b")

    fn = jax.jit(shard_map(
        fwd, mesh=mesh,
        in_specs=(P("b"), P()),          # x batch-sharded, params replicated
        out_specs=P("b"),
        check_rep=False,
    ))
    out = fn(x, params)
    return np.asarray(jax.device_get(out), dtype=np.float32)


def _run_cpu(x, params):
    cpu = jax.devices("cpu")[0]
    fwd = make_forward(axis_name=None)
    with jax.default_device(cpu):
        out = jax.jit(fwd)(jax.device_put(x, cpu),
                           jax.tree.map(lambda a: jax.device_put(a, cpu), params))
        return np.asarray(jax.device_get(out), dtype=np.float32)


def kernel(x, params):
    x = np.asarray(x, dtype=np.float32)
    params = _to_np_tree(params)

    if os.environ.get("COND_MBV2_FORCE_CPU", "") != "1":
        try:
            return _run_sharded_device(x, params)
        except Exception:
            pass
    return _run_cpu(x, params)


if __name__ == "__main__":
    rng = np.random.default_rng(0)
    x = rng.normal(size=(BATCH, 3, 224, 224)).astype(np.float32)
    print("kernel module self-check: arch specs:", block_specs()[0][:3], "...")
